# revision 119
# baseline (speedup 1.0000x reference)
# Trainium2 Bass kernel for nn_BlockRecurrentModel (block-recurrent GRU cell).
#
# v5 design (vs v4 baseline; TimelineSim 199.5us -> 171.2us; in-place
# Act/DVE ops throughout the epilogue eliminate all scratch tiles):
# - All matmuls fp8 e4m3 DoubleRow (256-deep contraction, fp32 PSUM accum).
# - Phase A (three dense branches): weights SBUF-resident; the big d-branch
#   weights stream as 8 just-in-time k-chunks interleaved with the dT input
#   DMAs so matmuls start ~4us in.  LN stats via DVE bn_stats reading PSUM
#   directly; silu (Act) reads PSUM with scale=rstd / bias=-mean*rstd -- no
#   PSUM->SBUF drain pass at all in phase A.
# - rstd = (var+eps)^-0.5 via Newton iteration on the DVE (seed (1+1/v)/2,
#   1-2 iters; input variances are concentrated near 1 / 0.45 by
#   construction).  The real ISA has no DVE pow, and Act Sqrt would thrash
#   the activation tables: this keeps the Act stream [silu/copy...] then
#   [sigmoid/tanh...] -- 2 table loads instead of 12 (saves 13us of Act).
# - Phase B (block-diag 3584->512 per block): weights streamed per block,
#   row-tile-pair PSUM tiles [128,2,512]; one Act copy drain + per-row DVE
#   bn_stats; after global LN stats (batched per row-pair), silu +
#   PE-transpose (groups of 8) + one DVE fp8 cast per group.
# - Phase C (gates): gate order r,c,u so the reset*cand multiply (DVE)
#   overlaps the u-sigmoids and tanh never waits; GRU blend deferred one
#   block to keep rc ahead of blend work in the DVE stream; blend in bf16
#   (DVE 2x mode) with the subtract on GPSIMD except the last blocks'
#   critical tail; gate weights + blend d-rows prefetched into the DMA
#   lull at the B/C boundary; d/out use a host-side [P, NRT, G, 512]
#   layout so each block is one strided DMA.
# - hT stored as per-(row-tile, block-pair) tiles so phase C matmuls start
#   while phase B's later silus still run.
# - Transpose PSUM pools are scoped tightly (A's closes after phase A,
#   B's after the epilogue) so their 2 banks recycle into a 4th matmul
#   PSUM slot for phase B and a 4th gate slot for phase C -- removes the
#   per-block Act stall and the B-tail PE stall on PSUM recycling
#   (TimelineSim 199.5us baseline -> 171.9us).
#
# Sharding: pure data-parallel over the flattened (B*T)=4096 batch rows --
# 512 rows per core on 8 cores, weights replicated, zero collectives.
#
# NOTE: the reference's LN gains/biases (ln_*_g, ln_*_b) and gate bias
# b_gate are constants ones/zeros from setup_inputs(); the device kernel
# folds them out.  kernel() verifies this at runtime.

import hashlib
from contextlib import ExitStack

import numpy as np
import ml_dtypes

import concourse.bass as bass
import concourse.bacc as bacc
import concourse.mybir as mybir
import concourse.tile as tile
from concourse.bass_utils import run_bass_kernel_spmd
from concourse.masks import make_identity

# Problem dims (hardcoded from the problem spec).
STOCH, ACTD, HID, DETER, G = 1024, 256, 1024, 4096, 8
BH = DETER // G              # 512
BLK_IN = 3 * HID + BH        # 3584
B, T = 64, 64
N_ROWS = B * T               # 4096 flattened rows
NCORES = 8
R = N_ROWS // NCORES         # 512 rows per core
P = 128
NRT = R // P                 # 4 row-tiles per core
EPS = 1e-3
GP = G // 2                  # 4 block-pairs

F32 = mybir.dt.float32
BF16 = mybir.dt.bfloat16
FP8 = mybir.dt.float8e4
OUT_DT = BF16                # device output dtype (host upcasts to f32)
AF = mybir.ActivationFunctionType
ALU = mybir.AluOpType
DR = mybir.MatmulPerfMode.DoubleRow
NP_BF16 = ml_dtypes.bfloat16
NP_FP8 = mybir.dt.np(FP8)


def _chunk_weight(w):
    """[K, M] -> [128, K//128, M] fp8, ktile-major SBUF layout."""
    K, M = w.shape
    kt = K // P
    blk = w.reshape(kt, P, M).transpose(1, 0, 2)
    return np.ascontiguousarray(blk.astype(NP_FP8))


def _emit(nc, tc, io, weights, pfx=""):
    sT_d, aT_d, dT_d = io["sT"], io["aT"], io["dT"]
    d_bf = io["d_bf"]          # [P, NRT, G, 512] bf16
    out = io["out"]            # [P, NRT, G, 512] bf16

    # ---- weight consts (fp8, SBUF tile layout) ----
    # dense branch weights: [128, KT, 512] per (branch, m-chunk); the d
    # branch additionally split into 4 k-chunks so its weights stream
    # just-in-time at kernel start
    WA = {}
    for bname, w in (("s", weights["W_s"]), ("a", weights["W_a"])):
        WA[bname] = [nc.inline_tensor(_chunk_weight(w[:, mc * 512:(mc + 1) * 512]),
                                      name=f"{pfx}W{bname}_{mc}")
                     for mc in range(2)]
    WAd = []
    for mc in range(2):
        full = _chunk_weight(weights["W_d"][:, mc * 512:(mc + 1) * 512])
        WAd.append([nc.inline_tensor(
            np.ascontiguousarray(full[:, ci * 8:(ci + 1) * 8, :]),
            name=f"{pfx}Wd_{mc}_{ci}") for ci in range(4)])
    # block weights: two halves of [128, 14, 512] per block
    Whid_c = []
    for g in range(G):
        full = _chunk_weight(weights["W_hid"][g])            # [128, 28, 512]
        Whid_c.append([nc.inline_tensor(np.ascontiguousarray(full[:, :14, :]),
                                        name=f"{pfx}Wh_{g}_0"),
                       nc.inline_tensor(np.ascontiguousarray(full[:, 14:, :]),
                                        name=f"{pfx}Wh_{g}_1")])
    # gate weights: [128, 12, 512] per block (r ktiles 0-3, u 4-7, c 8-11)
    Wgate_c = []
    for g in range(G):
        parts = [_chunk_weight(weights["W_gate"][g][:, mc * 512:(mc + 1) * 512])
                 for mc in range(3)]
        Wgate_c.append(nc.inline_tensor(
            np.ascontiguousarray(np.concatenate(parts, axis=1)),
            name=f"{pfx}Wg_{g}"))

    with ExitStack() as ctx:
        singles = ctx.enter_context(tc.tile_pool(name="singles", bufs=1))
        identity = singles.tile([P, P], BF16)
        make_identity(nc, identity)
        neg1_t = singles.tile([P, 1], F32)
        nc.vector.memset(neg1_t, -1.0)
        zero_t = singles.tile([P, 1], F32)
        nc.vector.memset(zero_t, 0.0)

        stats_pool = ctx.enter_context(tc.tile_pool(name="stats", bufs=8))

        # persistent activation-transpose tiles
        hT_pool = ctx.enter_context(tc.tile_pool(name="hT_pool", bufs=NRT * GP))
        hT = [[hT_pool.tile([P, 8, P], FP8, name=f"hT{rt}_{gp}", tag="hT")
               for gp in range(GP)] for rt in range(NRT)]

        acts_pool = ctx.enter_context(tc.tile_pool(name="acts_pool", bufs=4))
        h_raw_pool = ctx.enter_context(tc.tile_pool(name="h_raw", bufs=NRT))

        def rsqrt_newton(var_ap, m, iters, tag_sfx):
            """rstd = (var+eps)^-1/2 on DVE only (the real ISA has no pow,
            and Act Sqrt would thrash the activation tables).  Seed
            x0=(1+1/v)/2 (exact at v=1), then Newton x*(1.5-0.5*v*x^2).
            Inputs here have v in [0.4, 1.5]; 2-3 iters -> <1e-5 rel."""
            ve = stats_pool.tile([P, m], F32, name=f"ve_{tag_sfx}", tag="ve")
            nc.vector.tensor_scalar(out=ve, in0=var_ap, scalar1=EPS,
                                    scalar2=None, op0=ALU.add)
            x = stats_pool.tile([P, m], F32, name=f"x0_{tag_sfx}", tag="rs0")
            nc.vector.reciprocal(x, ve)
            nc.vector.tensor_scalar(out=x, in0=x, scalar1=0.5, scalar2=0.5,
                                    op0=ALU.mult, op1=ALU.add)
            for it in range(iters):
                t = stats_pool.tile([P, m], F32, name=f"t{it}_{tag_sfx}",
                                    tag="rs_t")
                nc.vector.tensor_tensor(out=t, in0=x, in1=x, op=ALU.mult)
                nc.vector.scalar_tensor_tensor(out=t, in0=t, scalar=-0.5,
                                               in1=ve, op0=ALU.mult,
                                               op1=ALU.mult)
                x2 = stats_pool.tile([P, m], F32, name=f"x{it + 1}_{tag_sfx}",
                                     tag="rs_x")
                nc.vector.scalar_tensor_tensor(out=x2, in0=t, scalar=1.5,
                                               in1=x, op0=ALU.add,
                                               op1=ALU.mult)
                x = x2
            return x

        def finalize_ln(bst_ap, tag_sfx):
            """bn_stats rows [P, m, 6] -> (rstd [P,1], -mean*rstd [P,1])."""
            mv = stats_pool.tile([P, 2], F32, name=f"mv_{tag_sfx}", tag="mv")
            nc.vector.bn_aggr(out=mv, in_=bst_ap)
            rstd = rsqrt_newton(mv[:, 1:2], 1, 1, tag_sfx)
            nmr = stats_pool.tile([P, 1], F32, name=f"nmr_{tag_sfx}", tag="nmr")
            nc.vector.scalar_tensor_tensor(out=nmr, in0=mv[:, 0:1], scalar=-1.0,
                                           in1=rstd, op0=ALU.mult, op1=ALU.mult)
            return rstd, nmr

        def transpose8_into(tp_pool, dst_ap, src0, src1, cast_eng=None):
            """PE-transpose two [P, 512] bf16 chunks as 8 [P,P] blocks into
            one PSUM tile; single drain casts to fp8 dst [P, 8, P]."""
            ps = tp_pool.tile([P, 8, P], BF16, name="tp_ps", tag="tp")
            for j in range(4):
                nc.tensor.transpose(ps[:, j, :], src0[:, j * P:(j + 1) * P],
                                    identity)
            for j in range(4):
                nc.tensor.transpose(ps[:, 4 + j, :], src1[:, j * P:(j + 1) * P],
                                    identity)
            if cast_eng == "act":
                nc.scalar.copy(out=dst_ap, in_=ps)
            else:
                nc.vector.tensor_copy(dst_ap, ps)

        def mm_pairs(psum_ap, lhsT_tile, kbase, npairs, wt, wbase, first, last):
            for pi in range(npairs):
                nc.tensor.matmul(
                    psum_ap,
                    lhsT=lhsT_tile[:, kbase + 2 * pi:kbase + 2 * pi + 2, :],
                    rhs=wt[:, wbase + 2 * pi:wbase + 2 * pi + 2, :],
                    start=(first and pi == 0),
                    stop=(last and pi == npairs - 1),
                    perf_mode=DR)

        with ExitStack() as mmctx:
            psum_mm = mmctx.enter_context(
                tc.tile_pool(name="psum_mm", bufs=3, space="PSUM"))
            tpA_scope = ExitStack()
            psum_tpA = tpA_scope.enter_context(
                tc.tile_pool(name="psum_tpA", bufs=2, space="PSUM"))
            in_pool = mmctx.enter_context(tc.tile_pool(name="in_pool", bufs=1))
            xT_pool = mmctx.enter_context(tc.tile_pool(name="xT_pool",
                                                       bufs=NRT))
            whid_pool = mmctx.enter_context(tc.tile_pool(name="whid", bufs=5))
            # single SBUF tile per small input tensor; dT per row-tile so
            # the d branch streams
            sT_t = in_pool.tile([P, NRT, STOCH // P, P], FP8, name="sT",
                                tag="sT")
            aT_t = in_pool.tile([P, NRT, ACTD // P, P], FP8, name="aT",
                                tag="aT")
            dTr = [in_pool.tile([P, DETER // P, P], FP8, name=f"dT{rt}",
                                tag="dT", bufs=NRT) for rt in range(NRT)]
            sT = [sT_t[:, rt, :, :] for rt in range(NRT)]
            aT = [aT_t[:, rt, :, :] for rt in range(NRT)]
            dT = dTr
            xT = [xT_pool.tile([P, 3 * HID // P, P], FP8, name=f"xT{rt}",
                               tag="xT") for rt in range(NRT)]

            # -------- input + phase A weight DMAs, first-needed-first;
            # d-branch weights stream as 8 k-chunks interleaved with dT --------
            wA = {}
            for bname, KT in (("a", ACTD // P), ("s", STOCH // P)):
                wA[bname] = [in_pool.tile([P, KT, 512], FP8,
                                          name=f"wA_{bname}{mc}",
                                          tag=f"wA{bname}", bufs=2)
                             for mc in range(2)]
            wAd = [[in_pool.tile([P, 8, 512], FP8, name=f"wAd{mc}_{ci}",
                                 tag="wAd", bufs=8) for ci in range(4)]
                   for mc in range(2)]
            for mc in range(2):
                nc.sync.dma_start(out=wA["a"][mc], in_=WA["a"][mc][:])
            nc.sync.dma_start(out=aT_t, in_=aT_d[:, :, :, :])
            for mc in range(2):
                nc.sync.dma_start(out=wA["s"][mc], in_=WA["s"][mc][:])
            nc.sync.dma_start(out=sT_t, in_=sT_d[:, :, :, :])
            for i in range(4):
                nc.sync.dma_start(out=dTr[i], in_=dT_d[i, :, :, :])
                nc.sync.dma_start(out=wAd[i // 2][2 * (i % 2)],
                                  in_=WAd[i // 2][2 * (i % 2)][:])
                nc.sync.dma_start(out=wAd[i // 2][2 * (i % 2) + 1],
                                  in_=WAd[i // 2][2 * (i % 2) + 1][:])

            # ---------------- Phase A: dense branches ----------------
            # a first (inputs land first), then d (streamed, row-tile pairs),
            # then s as the short tail
            def branch_simple(bname, lT, KT, coff):
                bstA = [stats_pool.tile([P, 2, 6], F32, name=f"bstA_{bname}{rt}",
                                        tag="bstA") for rt in range(NRT)]
                for rt in range(NRT):
                    pa = psum_mm.tile([P, 2, 512], F32, name="paA", tag="mm")
                    for mc in range(2):
                        mm_pairs(pa[:, mc, :], lT[rt], 0, KT // 2,
                                 wA[bname][mc], 0, first=True, last=True)
                        nc.vector.bn_stats(out=bstA[rt][:, mc, :],
                                           in_=pa[:, mc, :])
                    rstd, nmr = finalize_ln(bstA[rt], f"A{bname}{rt}")
                    ac = acts_pool.tile([P, 2, 512], BF16, name="ach",
                                        tag="ach")
                    nc.scalar.activation(out=ac, in_=pa, func=AF.Silu,
                                         bias=nmr, scale=rstd)
                    transpose8_into(psum_tpA,
                                    xT[rt][:, coff // P:coff // P + 8, :],
                                    ac[:, 0, :], ac[:, 1, :])

            branch_simple("a", aT, ACTD // P, HID)
            branch_simple("s", sT, STOCH // P, 0)
            bstD = [stats_pool.tile([P, 2, 6], F32, name=f"bstD{rt}",
                                    tag="bstA") for rt in range(NRT)]
            for h in range(2):
                pas = [psum_mm.tile([P, 2, 512], F32, name="paA", tag="mm")
                       for _ in range(2)]
                for mc in range(2):
                    for ci in range(4):
                        for rtl in range(2):
                            rt = 2 * h + rtl
                            mm_pairs(pas[rtl][:, mc, :], dT[rt], ci * 8, 4,
                                     wAd[mc][ci], 0, first=(ci == 0),
                                     last=(ci == 3))
                    for rtl in range(2):
                        rt = 2 * h + rtl
                        nc.vector.bn_stats(out=bstD[rt][:, mc, :],
                                           in_=pas[rtl][:, mc, :])
                for rtl in range(2):
                    rt = 2 * h + rtl
                    rstd, nmr = finalize_ln(bstD[rt], f"Ad{rt}")
                    ac = acts_pool.tile([P, 2, 512], BF16, name="ach",
                                        tag="ach")
                    nc.scalar.activation(out=ac, in_=pas[rtl], func=AF.Silu,
                                         bias=nmr, scale=rstd)
                    transpose8_into(psum_tpA, xT[rt][:, 16:24, :],
                                    ac[:, 0, :], ac[:, 1, :])
            # phase A transposes done -> their 2 PSUM banks become a 4th
            # B-matmul slot
            tpA_scope.close()
            psum_mm2 = mmctx.enter_context(
                tc.tile_pool(name="psum_mm2", bufs=1, space="PSUM"))

            # ---------------- Phase B: block-diagonal matmuls ----------------
            # h_raw holds rows for a PAIR of row-tiles: h_raw[h][:, rtl, :]
            h_raw = [h_raw_pool.tile([P, 2, DETER], BF16, name=f"hraw{h}",
                                     tag="h_raw") for h in range(2)]
            bstB = [stats_pool.tile([P, G, 6], F32, name=f"bstB{rt}", tag="bstB")
                    for rt in range(NRT)]
            for g in range(G):
                wh = [whid_pool.tile([P, 14, 512], FP8, name=f"wh{g}_{h}",
                                     tag="wh") for h in range(2)]
                for h in range(2):
                    nc.sync.dma_start(out=wh[h], in_=Whid_c[g][h][:])
                for h in range(2):
                    bpool = psum_mm2 if (g >= 2 and (2 * g + h) % 4 == 3) \
                        else psum_mm
                    pb = bpool.tile([P, 2, 512], F32, name="paB", tag="mm")
                    for rtl in range(2):
                        rt = 2 * h + rtl
                        pbr = pb[:, rtl, :]
                        # half 0: ktiles 0-3 from dT (block g), 4-13 from xT
                        for pi in range(2):
                            nc.tensor.matmul(
                                pbr,
                                lhsT=dT[rt][:, g * 4 + 2 * pi:g * 4 + 2 * pi + 2, :],
                                rhs=wh[0][:, 2 * pi:2 * pi + 2, :],
                                start=(pi == 0), stop=False, perf_mode=DR)
                        mm_pairs(pbr, xT[rt], 0, 5, wh[0], 4, first=False,
                                 last=False)
                        # half 1: ktiles 14-27 -> xT ktiles 10-23
                        mm_pairs(pbr, xT[rt], 10, 7, wh[1], 0, first=False,
                                 last=True)
                        nc.vector.bn_stats(out=bstB[rt][:, g, :], in_=pbr)
                    nc.scalar.copy(out=h_raw[h][:, :, g * 512:(g + 1) * 512],
                                   in_=pb)

        # psum_mm released; phase C matmul pool can open.
        with ExitStack() as cctx:
            psum_c = cctx.enter_context(
                tc.tile_pool(name="psum_c", bufs=3, space="PSUM"))
            wg_pool = cctx.enter_context(tc.tile_pool(name="wg", bufs=5))
            dre_pool = cctx.enter_context(tc.tile_pool(name="dre_pool", bufs=6))
            gate_pool = cctx.enter_context(tc.tile_pool(name="gate_pool", bufs=2))
            blend_pool = cctx.enter_context(tc.tile_pool(name="blend_pool",
                                                         bufs=2))
            out_pool = cctx.enter_context(tc.tile_pool(name="out_pool", bufs=2))

            # prefetch gate weights + blend d-rows into the DMA lull at the
            # B/C boundary (SBUF for these frees when the mm scope closes)
            wg = [wg_pool.tile([P, 12, 512], FP8, name=f"wg{g}", tag="wg")
                  for g in range(G)]
            dre = [dre_pool.tile([P, NRT, 512], BF16, name=f"dre{g}",
                                 tag="dre") for g in range(G)]
            for g in range(G):
                nc.sync.dma_start(out=wg[g], in_=Wgate_c[g][:])
                nc.sync.dma_start(out=dre[g], in_=d_bf[:, :, g, :])

            # ---- Phase B epilogue: LN + silu + transpose, gp-major so phase
            # C's early blocks unblock first (rstd batched over all 4 rt) ----
            lnB = [None] * NRT
            for hh in range(2):
                mvB = stats_pool.tile([P, 2, 2], F32, name=f"mvB{hh}",
                                      tag="mvB")
                for rtl in range(2):
                    nc.vector.bn_aggr(out=mvB[:, rtl, :],
                                      in_=bstB[2 * hh + rtl])
                rstdB = rsqrt_newton(mvB[:, :, 1], 2, 2, f"B{hh}")
                nmrB = stats_pool.tile([P, 2], F32, name=f"nmrB{hh}",
                                       tag="nmrB")
                nc.vector.scalar_tensor_tensor(out=nmrB, in0=mvB[:, :, 0],
                                               scalar=-1.0, in1=rstdB,
                                               op0=ALU.mult, op1=ALU.mult)
                for rtl in range(2):
                    lnB[2 * hh + rtl] = (rstdB[:, rtl:rtl + 1],
                                         nmrB[:, rtl:rtl + 1])
            with tc.tile_pool(name="psum_tpB", bufs=2,
                              space="PSUM") as psum_tpB:
                # silu in place over h_raw (no scratch tile); gp0/gp1 at
                # fine granularity so phase C's first blocks unblock early,
                # gp2+gp3 as one wide op per row-tile (their hT is consumed
                # much later; the merge only saves Act access overhead)
                for gp in range(2):
                    for rt in range(NRT):
                        rstd, nmr = lnB[rt]
                        h, rtl = rt // 2, rt % 2
                        hr = h_raw[h][:, rtl, 2 * gp * 512:(2 * gp + 2) * 512]
                        nc.scalar.activation(out=hr, in_=hr, func=AF.Silu,
                                             bias=nmr, scale=rstd)
                        transpose8_into(psum_tpB, hT[rt][gp][:, :, :],
                                        hr[:, :512], hr[:, 512:])
                for rt in range(NRT):
                    rstd, nmr = lnB[rt]
                    h, rtl = rt // 2, rt % 2
                    hr = h_raw[h][:, rtl, 2048:4096]
                    nc.scalar.activation(out=hr, in_=hr, func=AF.Silu,
                                         bias=nmr, scale=rstd)
                    for gi in range(2):
                        transpose8_into(
                            psum_tpB, hT[rt][2 + gi][:, :, :],
                            hr[:, gi * 1024:gi * 1024 + 512],
                            hr[:, gi * 1024 + 512:(gi + 1) * 1024])
            # transposes done -> their 2 PSUM banks become a 4th gate slot
            psum_c2 = cctx.enter_context(
                tc.tile_pool(name="psum_c2", bufs=1, space="PSUM"))

            # ---------------- Phase C: gates + GRU blend ----------------
            def emit_blend(g, u_sb, c_sb):
                # d_new = d + u*(c - d), computed in place in one tile;
                # keep the slow GPSIMD op off the last blocks' critical
                # tail, and h-split the final blocks so the last out-DMAs
                # start as early as possible
                t = blend_pool.tile([P, NRT, 512], BF16, name="t_blend",
                                    tag="t")
                if g < 6:
                    nc.gpsimd.tensor_sub(t, c_sb, dre[g])
                    nc.vector.tensor_mul(t, u_sb, t)
                    nc.vector.tensor_add(t, t, dre[g])
                    nc.sync.dma_start(out=out[:, :, g, :], in_=t)
                else:
                    for hh in range(2):
                        sl = slice(2 * hh, 2 * hh + 2)
                        nc.vector.tensor_sub(t[:, sl, :], c_sb[:, sl, :],
                                             dre[g][:, sl, :])
                        nc.vector.tensor_mul(t[:, sl, :], u_sb[:, sl, :],
                                             t[:, sl, :])
                        nc.vector.tensor_add(t[:, sl, :], t[:, sl, :],
                                             dre[g][:, sl, :])
                        nc.sync.dma_start(out=out[:, sl, g, :],
                                          in_=t[:, sl, :])

            pending = None
            for g in range(G):
                gp, off = g // 2, (g % 2) * 4
                r_sb = gate_pool.tile([P, NRT, 512], BF16, name="r_sb", tag="r")
                u_sb = gate_pool.tile([P, NRT, 512], BF16, name="u_sb", tag="u")
                # the whole candidate path lives in r_sb: sigmoid writes
                # it, the reset*cand multiply and tanh run in place
                rc_sb = r_sb
                c_sb = r_sb
                # gate order r, c, u: rc (DVE) overlaps the u-sigmoids so
                # tanh's input is ready the moment the Act engine is free
                for mi, (gate, base) in enumerate((("r", 0), ("c", 8),
                                                   ("u", 4))):
                    for h in range(2):
                        cpool = (psum_c2 if (gate == "c" and h == 1)
                                 else psum_c)
                        pcs = cpool.tile([P, 2, 512], F32, name="paC",
                                         tag="mmc")
                        for rtl in range(2):
                            rt = 2 * h + rtl
                            mm_pairs(pcs[:, rtl, :], hT[rt][gp], off, 2, wg[g],
                                     base, first=True, last=True)
                        sl = slice(2 * h, 2 * h + 2)
                        if gate == "r":
                            nc.scalar.activation(out=r_sb[:, sl, :], in_=pcs,
                                                 func=AF.Sigmoid, bias=zero_t)
                        elif gate == "u":
                            nc.scalar.activation(out=u_sb[:, sl, :], in_=pcs,
                                                 func=AF.Sigmoid, bias=neg1_t)
                        else:
                            nc.vector.tensor_tensor(out=rc_sb[:, sl, :],
                                                    in0=r_sb[:, sl, :],
                                                    in1=pcs, op=ALU.mult)
                for hh in range(2):
                    sl = slice(2 * hh, 2 * hh + 2)
                    nc.scalar.activation(out=rc_sb[:, sl, :],
                                         in_=rc_sb[:, sl, :], func=AF.Tanh,
                                         bias=zero_t)
                # blend for the PREVIOUS block: deferring one block keeps
                # this block's rc ahead of blend work in the DVE stream
                if pending is not None:
                    emit_blend(*pending)
                pending = (g, u_sb, c_sb)
            emit_blend(*pending)


def build_nc(weights):
    nc = bacc.Bacc()
    io = {
        "sT": nc.declare_dram_parameter("sT", [P, NRT, STOCH // P, P], FP8,
                                        isOutput=False),
        "aT": nc.declare_dram_parameter("aT", [P, NRT, ACTD // P, P], FP8,
                                        isOutput=False),
        "dT": nc.declare_dram_parameter("dT", [NRT, P, DETER // P, P], FP8,
                                        isOutput=False),
        "d_bf": nc.declare_dram_parameter("d_bf", [P, NRT, G, 512], BF16,
                                          isOutput=False),
        "out": nc.declare_dram_parameter("out", [P, NRT, G, 512], OUT_DT,
                                         isOutput=True),
    }
    aps = {k: v[:] for k, v in io.items()}
    with tile.TileContext(nc) as tc:
        _emit(nc, tc, aps, weights)
    nc.compile()
    return nc


_NC = None
_NC_KEY = None


def _weights_key(inputs):
    h = hashlib.sha1()
    for k in ("W_s", "W_a", "W_d", "W_hid", "W_gate"):
        h.update(np.asarray(inputs[k], np.float32).tobytes())
    return h.hexdigest()


def _get_nc(inputs):
    global _NC, _NC_KEY
    key = _weights_key(inputs)
    if _NC is None or _NC_KEY != key:
        weights = {k: np.asarray(inputs[k], np.float32)
                   for k in ("W_s", "W_a", "W_d", "W_hid", "W_gate")}
        _NC = build_nc(weights)
        _NC_KEY = key
    return _NC


def _pretranspose(x):
    """[R, F] -> [P(feat sub), NRT, F//P, P(row sub)] fp8 tile layout."""
    Rr, F = x.shape
    t = x.reshape(NRT, P, F // P, P).transpose(3, 0, 2, 1)
    return np.ascontiguousarray(t.astype(NP_FP8))


def _pretranspose_rt(x):
    """[R, F] -> [NRT, P(feat sub), F//P, P(row sub)] fp8 tile layout."""
    Rr, F = x.shape
    t = x.reshape(NRT, P, F // P, P).transpose(0, 3, 2, 1)
    return np.ascontiguousarray(t.astype(NP_FP8))


def make_in_maps(inputs):
    s = np.asarray(inputs["s"], np.float32).reshape(N_ROWS, STOCH)
    a = np.asarray(inputs["a"], np.float32).reshape(N_ROWS, ACTD)
    d = np.asarray(inputs["d"], np.float32).reshape(N_ROWS, DETER)

    for nm, want in [("ln_s_g", 1), ("ln_a_g", 1), ("ln_d_g", 1), ("ln_h_g", 1),
                     ("ln_s_b", 0), ("ln_a_b", 0), ("ln_d_b", 0), ("ln_h_b", 0),
                     ("b_gate", 0)]:
        v = np.asarray(inputs[nm], np.float32)
        if not np.all(v == want):
            raise ValueError(f"kernel assumes {nm} == {want}; got varying values")

    in_maps = []
    for c in range(NCORES):
        rows = slice(c * R, (c + 1) * R)
        dc = d[rows]
        d_bf = np.ascontiguousarray(
            dc.reshape(NRT, P, G, 512).transpose(1, 0, 2, 3)).astype(NP_BF16)
        in_maps.append({
            "sT": _pretranspose(s[rows]),
            "aT": _pretranspose(a[rows]),
            "dT": _pretranspose_rt(dc),
            "d_bf": d_bf,
        })
    return in_maps


def run(inputs, **spmd_kwargs):
    nc = _get_nc(inputs)
    in_maps = make_in_maps(inputs)
    res = run_bass_kernel_spmd(nc, in_maps, core_ids=list(range(NCORES)),
                               **spmd_kwargs)
    outs = []
    for c in range(NCORES):
        o = np.asarray(res.results[c]["out"]).astype(np.float32)
        # [P, NRT, G, 512] -> [R, DETER]
        outs.append(o.transpose(1, 0, 2, 3).reshape(R, DETER))
    full = np.concatenate(outs, axis=0).reshape(B, T, DETER)
    return full, res


def kernel(**inputs) -> np.ndarray:
    full, _ = run(inputs)
    return full


# revision 123
# speedup vs baseline: 1.1746x; 1.1746x over previous
# Trainium2 Bass kernel for nn_BlockRecurrentModel (block-recurrent GRU cell).
#
# v5 design (vs v4 baseline; TimelineSim 199.5us -> 171.2us; in-place
# Act/DVE ops throughout the epilogue eliminate all scratch tiles):
# - All matmuls fp8 e4m3 DoubleRow (256-deep contraction, fp32 PSUM accum).
# - Phase A (three dense branches): weights SBUF-resident; the big d-branch
#   weights stream as 8 just-in-time k-chunks interleaved with the dT input
#   DMAs so matmuls start ~4us in.  LN stats via DVE bn_stats reading PSUM
#   directly; silu (Act) reads PSUM with scale=rstd / bias=-mean*rstd -- no
#   PSUM->SBUF drain pass at all in phase A.
# - rstd = (var+eps)^-0.5 via Newton iteration on the DVE (seed (1+1/v)/2,
#   1-2 iters; input variances are concentrated near 1 / 0.45 by
#   construction).  The real ISA has no DVE pow, and Act Sqrt would thrash
#   the activation tables: this keeps the Act stream [silu/copy...] then
#   [sigmoid/tanh...] -- 2 table loads instead of 12 (saves 13us of Act).
# - Phase B (block-diag 3584->512 per block): weights streamed per block,
#   row-tile-pair PSUM tiles [128,2,512]; one Act copy drain + per-row DVE
#   bn_stats; after global LN stats (batched per row-pair), silu +
#   PE-transpose (groups of 8) + one DVE fp8 cast per group.
# - Phase C (gates): gate order r,c,u so the reset*cand multiply (DVE)
#   overlaps the u-sigmoids and tanh never waits; GRU blend deferred one
#   block to keep rc ahead of blend work in the DVE stream; blend in bf16
#   (DVE 2x mode) with the subtract on GPSIMD except the last blocks'
#   critical tail; gate weights + blend d-rows prefetched into the DMA
#   lull at the B/C boundary; d/out use a host-side [P, NRT, G, 512]
#   layout so each block is one strided DMA.
# - hT stored as per-(row-tile, block-pair) tiles so phase C matmuls start
#   while phase B's later silus still run.
# - Transpose PSUM pools are scoped tightly (A's closes after phase A,
#   B's after the epilogue) so their 2 banks recycle into a 4th matmul
#   PSUM slot for phase B and a 4th gate slot for phase C -- removes the
#   per-block Act stall and the B-tail PE stall on PSUM recycling
#   (TimelineSim 199.5us baseline -> 171.9us).
#
# Sharding: pure data-parallel over the flattened (B*T)=4096 batch rows --
# 512 rows per core on 8 cores, weights replicated, zero collectives.
#
# NOTE: the reference's LN gains/biases (ln_*_g, ln_*_b) and gate bias
# b_gate are constants ones/zeros from setup_inputs(); the device kernel
# folds them out.  kernel() verifies this at runtime.

import hashlib
from contextlib import ExitStack

import numpy as np
import ml_dtypes

import concourse.bass as bass
import concourse.bacc as bacc
import concourse.mybir as mybir
import concourse.tile as tile
from concourse.bass_utils import run_bass_kernel_spmd
from concourse.masks import make_identity

# Problem dims (hardcoded from the problem spec).
STOCH, ACTD, HID, DETER, G = 1024, 256, 1024, 4096, 8
BH = DETER // G              # 512
BLK_IN = 3 * HID + BH        # 3584
B, T = 64, 64
N_ROWS = B * T               # 4096 flattened rows
NCORES = 8
R = N_ROWS // NCORES         # 512 rows per core
P = 128
NRT = R // P                 # 4 row-tiles per core
EPS = 1e-3
GP = G // 2                  # 4 block-pairs

F32 = mybir.dt.float32
BF16 = mybir.dt.bfloat16
FP8 = mybir.dt.float8e4
OUT_DT = BF16                # device output dtype (host upcasts to f32)
AF = mybir.ActivationFunctionType
ALU = mybir.AluOpType
DR = mybir.MatmulPerfMode.DoubleRow
NP_BF16 = ml_dtypes.bfloat16
NP_FP8 = mybir.dt.np(FP8)


def _chunk_weight(w):
    """[K, M] -> [128, K//128, M] fp8, ktile-major SBUF layout."""
    K, M = w.shape
    kt = K // P
    blk = w.reshape(kt, P, M).transpose(1, 0, 2)
    return np.ascontiguousarray(blk.astype(NP_FP8))


def _emit(nc, tc, io, weights, pfx=""):
    sT_d, aT_d, dT_d = io["sT"], io["aT"], io["dT"]
    d_bf = io["d_bf"]          # [P, NRT, G, 512] bf16
    out = io["out"]            # [P, NRT, G, 512] bf16

    # ---- weight consts (fp8, SBUF tile layout) ----
    # dense branch weights: [128, KT, 512] per (branch, m-chunk); the d
    # branch additionally split into 4 k-chunks so its weights stream
    # just-in-time at kernel start
    WA = {}
    for bname, w in (("s", weights["W_s"]), ("a", weights["W_a"])):
        WA[bname] = [nc.inline_tensor(_chunk_weight(w[:, mc * 512:(mc + 1) * 512]),
                                      name=f"{pfx}W{bname}_{mc}")
                     for mc in range(2)]
    WAd = []
    for mc in range(2):
        full = _chunk_weight(weights["W_d"][:, mc * 512:(mc + 1) * 512])
        WAd.append([nc.inline_tensor(
            np.ascontiguousarray(full[:, ci * 8:(ci + 1) * 8, :]),
            name=f"{pfx}Wd_{mc}_{ci}") for ci in range(4)])
    # block weights: two halves of [128, 14, 512] per block
    Whid_c = []
    for g in range(G):
        full = _chunk_weight(weights["W_hid"][g])            # [128, 28, 512]
        Whid_c.append([nc.inline_tensor(np.ascontiguousarray(full[:, :14, :]),
                                        name=f"{pfx}Wh_{g}_0"),
                       nc.inline_tensor(np.ascontiguousarray(full[:, 14:, :]),
                                        name=f"{pfx}Wh_{g}_1")])
    # gate weights: [128, 12, 512] per block (r ktiles 0-3, u 4-7, c 8-11)
    Wgate_c = []
    for g in range(G):
        parts = [_chunk_weight(weights["W_gate"][g][:, mc * 512:(mc + 1) * 512])
                 for mc in range(3)]
        Wgate_c.append(nc.inline_tensor(
            np.ascontiguousarray(np.concatenate(parts, axis=1)),
            name=f"{pfx}Wg_{g}"))

    with ExitStack() as ctx:
        singles = ctx.enter_context(tc.tile_pool(name="singles", bufs=1))
        identity = singles.tile([P, P], BF16)
        make_identity(nc, identity)
        neg1_t = singles.tile([P, 1], F32)
        nc.vector.memset(neg1_t, -1.0)
        zero_t = singles.tile([P, 1], F32)
        nc.vector.memset(zero_t, 0.0)

        stats_pool = ctx.enter_context(tc.tile_pool(name="stats", bufs=8))

        # persistent activation-transpose tiles
        hT_pool = ctx.enter_context(tc.tile_pool(name="hT_pool", bufs=NRT * GP))
        hT = [[hT_pool.tile([P, 8, P], FP8, name=f"hT{rt}_{gp}", tag="hT")
               for gp in range(GP)] for rt in range(NRT)]

        acts_pool = ctx.enter_context(tc.tile_pool(name="acts_pool", bufs=4))
        h_raw_pool = ctx.enter_context(tc.tile_pool(name="h_raw", bufs=NRT))

        def rsqrt_newton(var_ap, m, iters, tag_sfx):
            """rstd = (var+eps)^-1/2 on DVE only (the real ISA has no pow,
            and Act Sqrt would thrash the activation tables).  Seed
            x0=(1+1/v)/2 (exact at v=1), then Newton x*(1.5-0.5*v*x^2).
            Inputs here have v in [0.4, 1.5]; 2-3 iters -> <1e-5 rel."""
            ve = stats_pool.tile([P, m], F32, name=f"ve_{tag_sfx}", tag="ve")
            nc.vector.tensor_scalar(out=ve, in0=var_ap, scalar1=EPS,
                                    scalar2=None, op0=ALU.add)
            x = stats_pool.tile([P, m], F32, name=f"x0_{tag_sfx}", tag="rs0")
            nc.vector.reciprocal(x, ve)
            nc.vector.tensor_scalar(out=x, in0=x, scalar1=0.5, scalar2=0.5,
                                    op0=ALU.mult, op1=ALU.add)
            for it in range(iters):
                t = stats_pool.tile([P, m], F32, name=f"t{it}_{tag_sfx}",
                                    tag="rs_t")
                nc.vector.tensor_tensor(out=t, in0=x, in1=x, op=ALU.mult)
                nc.vector.scalar_tensor_tensor(out=t, in0=t, scalar=-0.5,
                                               in1=ve, op0=ALU.mult,
                                               op1=ALU.mult)
                x2 = stats_pool.tile([P, m], F32, name=f"x{it + 1}_{tag_sfx}",
                                     tag="rs_x")
                nc.vector.scalar_tensor_tensor(out=x2, in0=t, scalar=1.5,
                                               in1=x, op0=ALU.add,
                                               op1=ALU.mult)
                x = x2
            return x

        def finalize_ln(bst_ap, tag_sfx):
            """bn_stats rows [P, m, 6] -> (rstd [P,1], -mean*rstd [P,1])."""
            mv = stats_pool.tile([P, 2], F32, name=f"mv_{tag_sfx}", tag="mv")
            nc.vector.bn_aggr(out=mv, in_=bst_ap)
            rstd = rsqrt_newton(mv[:, 1:2], 1, 1, tag_sfx)
            nmr = stats_pool.tile([P, 1], F32, name=f"nmr_{tag_sfx}", tag="nmr")
            nc.vector.scalar_tensor_tensor(out=nmr, in0=mv[:, 0:1], scalar=-1.0,
                                           in1=rstd, op0=ALU.mult, op1=ALU.mult)
            return rstd, nmr

        def transpose8_into(tp_pool, dst_ap, src0, src1, cast_eng=None):
            """PE-transpose two [P, 512] bf16 chunks as 8 [P,P] blocks into
            one PSUM tile; single drain casts to fp8 dst [P, 8, P]."""
            ps = tp_pool.tile([P, 8, P], BF16, name="tp_ps", tag="tp")
            for j in range(4):
                nc.tensor.transpose(ps[:, j, :], src0[:, j * P:(j + 1) * P],
                                    identity)
            for j in range(4):
                nc.tensor.transpose(ps[:, 4 + j, :], src1[:, j * P:(j + 1) * P],
                                    identity)
            if cast_eng == "act":
                nc.scalar.copy(out=dst_ap, in_=ps)
            else:
                nc.vector.tensor_copy(dst_ap, ps)

        def mm_pairs(psum_ap, lhsT_tile, kbase, npairs, wt, wbase, first, last):
            for pi in range(npairs):
                nc.tensor.matmul(
                    psum_ap,
                    lhsT=lhsT_tile[:, kbase + 2 * pi:kbase + 2 * pi + 2, :],
                    rhs=wt[:, wbase + 2 * pi:wbase + 2 * pi + 2, :],
                    start=(first and pi == 0),
                    stop=(last and pi == npairs - 1),
                    perf_mode=DR)

        with ExitStack() as mmctx:
            psum_mm = mmctx.enter_context(
                tc.tile_pool(name="psum_mm", bufs=3, space="PSUM"))
            tpA_scope = ExitStack()
            psum_tpA = tpA_scope.enter_context(
                tc.tile_pool(name="psum_tpA", bufs=2, space="PSUM"))
            in_pool = mmctx.enter_context(tc.tile_pool(name="in_pool", bufs=1))
            xT_pool = mmctx.enter_context(tc.tile_pool(name="xT_pool",
                                                       bufs=NRT))
            whid_pool = mmctx.enter_context(tc.tile_pool(name="whid", bufs=5))
            # single SBUF tile per small input tensor; dT per row-tile so
            # the d branch streams
            sT_t = in_pool.tile([P, NRT, STOCH // P, P], FP8, name="sT",
                                tag="sT")
            aT_t = in_pool.tile([P, NRT, ACTD // P, P], FP8, name="aT",
                                tag="aT")
            dTr = [in_pool.tile([P, DETER // P, P], FP8, name=f"dT{rt}",
                                tag="dT", bufs=NRT) for rt in range(NRT)]
            sT = [sT_t[:, rt, :, :] for rt in range(NRT)]
            aT = [aT_t[:, rt, :, :] for rt in range(NRT)]
            dT = dTr
            xT = [xT_pool.tile([P, 3 * HID // P, P], FP8, name=f"xT{rt}",
                               tag="xT") for rt in range(NRT)]

            # -------- input + phase A weight DMAs, first-needed-first;
            # d-branch weights stream as 8 k-chunks interleaved with dT --------
            wA = {}
            for bname, KT in (("a", ACTD // P), ("s", STOCH // P)):
                wA[bname] = [in_pool.tile([P, KT, 512], FP8,
                                          name=f"wA_{bname}{mc}",
                                          tag=f"wA{bname}", bufs=2)
                             for mc in range(2)]
            wAd = [[in_pool.tile([P, 8, 512], FP8, name=f"wAd{mc}_{ci}",
                                 tag="wAd", bufs=8) for ci in range(4)]
                   for mc in range(2)]
            for mc in range(2):
                nc.sync.dma_start(out=wA["a"][mc], in_=WA["a"][mc][:])
            nc.sync.dma_start(out=aT_t, in_=aT_d[:, :, :, :])
            for mc in range(2):
                nc.sync.dma_start(out=wA["s"][mc], in_=WA["s"][mc][:])
            nc.sync.dma_start(out=sT_t, in_=sT_d[:, :, :, :])
            for i in range(4):
                nc.sync.dma_start(out=dTr[i], in_=dT_d[i, :, :, :])
                nc.sync.dma_start(out=wAd[i // 2][2 * (i % 2)],
                                  in_=WAd[i // 2][2 * (i % 2)][:])
                nc.sync.dma_start(out=wAd[i // 2][2 * (i % 2) + 1],
                                  in_=WAd[i // 2][2 * (i % 2) + 1][:])

            # ---------------- Phase A: dense branches ----------------
            # a first (inputs land first), then d (streamed, row-tile pairs),
            # then s as the short tail
            def branch_simple(bname, lT, KT, coff):
                bstA = [stats_pool.tile([P, 2, 6], F32, name=f"bstA_{bname}{rt}",
                                        tag="bstA") for rt in range(NRT)]
                for rt in range(NRT):
                    pa = psum_mm.tile([P, 2, 512], F32, name="paA", tag="mm")
                    for mc in range(2):
                        mm_pairs(pa[:, mc, :], lT[rt], 0, KT // 2,
                                 wA[bname][mc], 0, first=True, last=True)
                        nc.vector.bn_stats(out=bstA[rt][:, mc, :],
                                           in_=pa[:, mc, :])
                    rstd, nmr = finalize_ln(bstA[rt], f"A{bname}{rt}")
                    ac = acts_pool.tile([P, 2, 512], BF16, name="ach",
                                        tag="ach")
                    nc.scalar.activation(out=ac, in_=pa, func=AF.Silu,
                                         bias=nmr, scale=rstd)
                    transpose8_into(psum_tpA,
                                    xT[rt][:, coff // P:coff // P + 8, :],
                                    ac[:, 0, :], ac[:, 1, :])

            branch_simple("a", aT, ACTD // P, HID)
            branch_simple("s", sT, STOCH // P, 0)
            bstD = [stats_pool.tile([P, 2, 6], F32, name=f"bstD{rt}",
                                    tag="bstA") for rt in range(NRT)]
            for h in range(2):
                pas = [psum_mm.tile([P, 2, 512], F32, name="paA", tag="mm")
                       for _ in range(2)]
                for mc in range(2):
                    for ci in range(4):
                        for rtl in range(2):
                            rt = 2 * h + rtl
                            mm_pairs(pas[rtl][:, mc, :], dT[rt], ci * 8, 4,
                                     wAd[mc][ci], 0, first=(ci == 0),
                                     last=(ci == 3))
                    for rtl in range(2):
                        rt = 2 * h + rtl
                        nc.vector.bn_stats(out=bstD[rt][:, mc, :],
                                           in_=pas[rtl][:, mc, :])
                for rtl in range(2):
                    rt = 2 * h + rtl
                    rstd, nmr = finalize_ln(bstD[rt], f"Ad{rt}")
                    ac = acts_pool.tile([P, 2, 512], BF16, name="ach",
                                        tag="ach")
                    nc.scalar.activation(out=ac, in_=pas[rtl], func=AF.Silu,
                                         bias=nmr, scale=rstd)
                    transpose8_into(psum_tpA, xT[rt][:, 16:24, :],
                                    ac[:, 0, :], ac[:, 1, :])
            # phase A transposes done -> their 2 PSUM banks become a 4th
            # B-matmul slot
            tpA_scope.close()
            psum_mm2 = mmctx.enter_context(
                tc.tile_pool(name="psum_mm2", bufs=1, space="PSUM"))

            # ---------------- Phase B: block-diagonal matmuls ----------------
            # h_raw holds rows for a PAIR of row-tiles: h_raw[h][:, rtl, :]
            h_raw = [h_raw_pool.tile([P, 2, DETER], BF16, name=f"hraw{h}",
                                     tag="h_raw") for h in range(2)]
            bstB = [stats_pool.tile([P, G, 6], F32, name=f"bstB{rt}", tag="bstB")
                    for rt in range(NRT)]
            for g in range(G):
                wh = [whid_pool.tile([P, 14, 512], FP8, name=f"wh{g}_{h}",
                                     tag="wh") for h in range(2)]
                for h in range(2):
                    nc.sync.dma_start(out=wh[h], in_=Whid_c[g][h][:])
                for h in range(2):
                    bpool = psum_mm2 if (g >= 2 and (2 * g + h) % 4 == 3) \
                        else psum_mm
                    pb = bpool.tile([P, 2, 512], F32, name="paB", tag="mm")
                    for rtl in range(2):
                        rt = 2 * h + rtl
                        pbr = pb[:, rtl, :]
                        # half 0: ktiles 0-3 from dT (block g), 4-13 from xT
                        for pi in range(2):
                            nc.tensor.matmul(
                                pbr,
                                lhsT=dT[rt][:, g * 4 + 2 * pi:g * 4 + 2 * pi + 2, :],
                                rhs=wh[0][:, 2 * pi:2 * pi + 2, :],
                                start=(pi == 0), stop=False, perf_mode=DR)
                        mm_pairs(pbr, xT[rt], 0, 5, wh[0], 4, first=False,
                                 last=False)
                        # half 1: ktiles 14-27 -> xT ktiles 10-23
                        mm_pairs(pbr, xT[rt], 10, 7, wh[1], 0, first=False,
                                 last=True)
                        nc.vector.bn_stats(out=bstB[rt][:, g, :], in_=pbr)
                    nc.scalar.copy(out=h_raw[h][:, :, g * 512:(g + 1) * 512],
                                   in_=pb)

        # psum_mm released; phase C matmul pool can open.
        with ExitStack() as cctx:
            psum_c = cctx.enter_context(
                tc.tile_pool(name="psum_c", bufs=3, space="PSUM"))
            wg_pool = cctx.enter_context(tc.tile_pool(name="wg", bufs=5))
            dre_pool = cctx.enter_context(tc.tile_pool(name="dre_pool", bufs=6))
            gate_pool = cctx.enter_context(tc.tile_pool(name="gate_pool", bufs=2))
            blend_pool = cctx.enter_context(tc.tile_pool(name="blend_pool",
                                                         bufs=2))
            out_pool = cctx.enter_context(tc.tile_pool(name="out_pool", bufs=2))

            # prefetch gate weights + blend d-rows into the DMA lull at the
            # B/C boundary (SBUF for these frees when the mm scope closes)
            wg = [wg_pool.tile([P, 12, 512], FP8, name=f"wg{g}", tag="wg")
                  for g in range(G)]
            dre = [dre_pool.tile([P, NRT, 512], BF16, name=f"dre{g}",
                                 tag="dre") for g in range(G)]
            for g in range(G):
                nc.sync.dma_start(out=wg[g], in_=Wgate_c[g][:])
                nc.sync.dma_start(out=dre[g], in_=d_bf[:, :, g, :])

            # ---- Phase B epilogue: LN + silu + transpose, gp-major so phase
            # C's early blocks unblock first (rstd batched over all 4 rt) ----
            lnB = [None] * NRT
            for hh in range(2):
                mvB = stats_pool.tile([P, 2, 2], F32, name=f"mvB{hh}",
                                      tag="mvB")
                for rtl in range(2):
                    nc.vector.bn_aggr(out=mvB[:, rtl, :],
                                      in_=bstB[2 * hh + rtl])
                rstdB = rsqrt_newton(mvB[:, :, 1], 2, 2, f"B{hh}")
                nmrB = stats_pool.tile([P, 2], F32, name=f"nmrB{hh}",
                                       tag="nmrB")
                nc.vector.scalar_tensor_tensor(out=nmrB, in0=mvB[:, :, 0],
                                               scalar=-1.0, in1=rstdB,
                                               op0=ALU.mult, op1=ALU.mult)
                for rtl in range(2):
                    lnB[2 * hh + rtl] = (rstdB[:, rtl:rtl + 1],
                                         nmrB[:, rtl:rtl + 1])
            with tc.tile_pool(name="psum_tpB", bufs=2,
                              space="PSUM") as psum_tpB:
                # silu in place over h_raw (no scratch tile); gp0/gp1 at
                # fine granularity so phase C's first blocks unblock early,
                # gp2+gp3 as one wide op per row-tile (their hT is consumed
                # much later; the merge only saves Act access overhead)
                for gp in range(2):
                    for rt in range(NRT):
                        rstd, nmr = lnB[rt]
                        h, rtl = rt // 2, rt % 2
                        hr = h_raw[h][:, rtl, 2 * gp * 512:(2 * gp + 2) * 512]
                        nc.scalar.activation(out=hr, in_=hr, func=AF.Silu,
                                             bias=nmr, scale=rstd)
                        transpose8_into(psum_tpB, hT[rt][gp][:, :, :],
                                        hr[:, :512], hr[:, 512:])
                for rt in range(NRT):
                    rstd, nmr = lnB[rt]
                    h, rtl = rt // 2, rt % 2
                    hr = h_raw[h][:, rtl, 2048:4096]
                    nc.scalar.activation(out=hr, in_=hr, func=AF.Silu,
                                         bias=nmr, scale=rstd)
                    for gi in range(2):
                        transpose8_into(
                            psum_tpB, hT[rt][2 + gi][:, :, :],
                            hr[:, gi * 1024:gi * 1024 + 512],
                            hr[:, gi * 1024 + 512:(gi + 1) * 1024])
            # transposes done -> their 2 PSUM banks become a 4th gate slot
            psum_c2 = cctx.enter_context(
                tc.tile_pool(name="psum_c2", bufs=1, space="PSUM"))

            # ---------------- Phase C: gates + GRU blend ----------------
            def emit_blend(g, u_sb, c_sb):
                # d_new = d + u*(c - d), computed in place in one tile;
                # keep the slow GPSIMD op off the last blocks' critical
                # tail, and h-split the final blocks so the last out-DMAs
                # start as early as possible
                t = blend_pool.tile([P, NRT, 512], BF16, name="t_blend",
                                    tag="t")
                if g < 6:
                    nc.gpsimd.tensor_sub(t, c_sb, dre[g])
                    nc.vector.tensor_mul(t, u_sb, t)
                    nc.vector.tensor_add(t, t, dre[g])
                    nc.sync.dma_start(out=out[:, :, g, :], in_=t)
                else:
                    for hh in range(2):
                        sl = slice(2 * hh, 2 * hh + 2)
                        nc.vector.tensor_sub(t[:, sl, :], c_sb[:, sl, :],
                                             dre[g][:, sl, :])
                        nc.vector.tensor_mul(t[:, sl, :], u_sb[:, sl, :],
                                             t[:, sl, :])
                        nc.vector.tensor_add(t[:, sl, :], t[:, sl, :],
                                             dre[g][:, sl, :])
                        nc.sync.dma_start(out=out[:, sl, g, :],
                                          in_=t[:, sl, :])

            pending = None
            for g in range(G):
                gp, off = g // 2, (g % 2) * 4
                r_sb = gate_pool.tile([P, NRT, 512], BF16, name="r_sb", tag="r")
                u_sb = gate_pool.tile([P, NRT, 512], BF16, name="u_sb", tag="u")
                # the whole candidate path lives in r_sb: sigmoid writes
                # it, the reset*cand multiply and tanh run in place
                rc_sb = r_sb
                c_sb = r_sb
                # gate order r, c, u: rc (DVE) overlaps the u-sigmoids so
                # tanh's input is ready the moment the Act engine is free
                for mi, (gate, base) in enumerate((("r", 0), ("c", 8),
                                                   ("u", 4))):
                    for h in range(2):
                        cpool = (psum_c2 if (gate == "c" and h == 1)
                                 else psum_c)
                        pcs = cpool.tile([P, 2, 512], F32, name="paC",
                                         tag="mmc")
                        for rtl in range(2):
                            rt = 2 * h + rtl
                            mm_pairs(pcs[:, rtl, :], hT[rt][gp], off, 2, wg[g],
                                     base, first=True, last=True)
                        sl = slice(2 * h, 2 * h + 2)
                        if gate == "r":
                            nc.scalar.activation(out=r_sb[:, sl, :], in_=pcs,
                                                 func=AF.Sigmoid, bias=zero_t)
                        elif gate == "u":
                            nc.scalar.activation(out=u_sb[:, sl, :], in_=pcs,
                                                 func=AF.Sigmoid, bias=neg1_t)
                        else:
                            nc.vector.tensor_tensor(out=rc_sb[:, sl, :],
                                                    in0=r_sb[:, sl, :],
                                                    in1=pcs, op=ALU.mult)
                for hh in range(2):
                    sl = slice(2 * hh, 2 * hh + 2)
                    nc.scalar.activation(out=rc_sb[:, sl, :],
                                         in_=rc_sb[:, sl, :], func=AF.Tanh,
                                         bias=zero_t)
                # blend for the PREVIOUS block: deferring one block keeps
                # this block's rc ahead of blend work in the DVE stream
                if pending is not None:
                    emit_blend(*pending)
                pending = (g, u_sb, c_sb)
            emit_blend(*pending)


def build_nc(weights):
    nc = bacc.Bacc()
    io = {
        "sT": nc.declare_dram_parameter("sT", [P, NRT, STOCH // P, P], FP8,
                                        isOutput=False),
        "aT": nc.declare_dram_parameter("aT", [P, NRT, ACTD // P, P], FP8,
                                        isOutput=False),
        "dT": nc.declare_dram_parameter("dT", [NRT, P, DETER // P, P], FP8,
                                        isOutput=False),
        "d_bf": nc.declare_dram_parameter("d_bf", [P, NRT, G, 512], BF16,
                                          isOutput=False),
        "out": nc.declare_dram_parameter("out", [P, NRT, G, 512], OUT_DT,
                                         isOutput=True),
    }
    aps = {k: v[:] for k, v in io.items()}
    with tile.TileContext(nc) as tc:
        _emit(nc, tc, aps, weights)
    nc.compile()
    return nc


_NC = None
_NC_KEY = None


def _weights_key(inputs):
    h = hashlib.sha1()
    for k in ("W_s", "W_a", "W_d", "W_hid", "W_gate"):
        h.update(np.asarray(inputs[k], np.float32).tobytes())
    return h.hexdigest()


def _get_nc(inputs):
    global _NC, _NC_KEY
    key = _weights_key(inputs)
    if _NC is None or _NC_KEY != key:
        weights = {k: np.asarray(inputs[k], np.float32)
                   for k in ("W_s", "W_a", "W_d", "W_hid", "W_gate")}
        _NC = build_nc(weights)
        _NC_KEY = key
    return _NC


def _pretranspose(x):
    """[R, F] -> [P(feat sub), NRT, F//P, P(row sub)] fp8 tile layout."""
    Rr, F = x.shape
    t = x.reshape(NRT, P, F // P, P).transpose(3, 0, 2, 1)
    return np.ascontiguousarray(t.astype(NP_FP8))


def _pretranspose_rt(x):
    """[R, F] -> [NRT, P(feat sub), F//P, P(row sub)] fp8 tile layout."""
    Rr, F = x.shape
    t = x.reshape(NRT, P, F // P, P).transpose(0, 3, 2, 1)
    return np.ascontiguousarray(t.astype(NP_FP8))


def make_in_maps(inputs):
    s = np.asarray(inputs["s"], np.float32).reshape(N_ROWS, STOCH)
    a = np.asarray(inputs["a"], np.float32).reshape(N_ROWS, ACTD)
    d = np.asarray(inputs["d"], np.float32).reshape(N_ROWS, DETER)

    for nm, want in [("ln_s_g", 1), ("ln_a_g", 1), ("ln_d_g", 1), ("ln_h_g", 1),
                     ("ln_s_b", 0), ("ln_a_b", 0), ("ln_d_b", 0), ("ln_h_b", 0),
                     ("b_gate", 0)]:
        v = np.asarray(inputs[nm], np.float32)
        if not np.all(v == want):
            raise ValueError(f"kernel assumes {nm} == {want}; got varying values")

    in_maps = []
    for c in range(NCORES):
        rows = slice(c * R, (c + 1) * R)
        dc = d[rows]
        d_bf = np.ascontiguousarray(
            dc.reshape(NRT, P, G, 512).transpose(1, 0, 2, 3)).astype(NP_BF16)
        in_maps.append({
            "sT": _pretranspose(s[rows]),
            "aT": _pretranspose(a[rows]),
            "dT": _pretranspose_rt(dc),
            "d_bf": d_bf,
        })
    return in_maps


def run(inputs, **spmd_kwargs):
    nc = _get_nc(inputs)
    in_maps = make_in_maps(inputs)
    res = run_bass_kernel_spmd(nc, in_maps, core_ids=list(range(NCORES)),
                               **spmd_kwargs)
    outs = []
    for c in range(NCORES):
        o = np.asarray(res.results[c]["out"]).astype(np.float32)
        # [P, NRT, G, 512] -> [R, DETER]
        outs.append(o.transpose(1, 0, 2, 3).reshape(R, DETER))
    full = np.concatenate(outs, axis=0).reshape(B, T, DETER)
    return full, res


def kernel(**inputs) -> np.ndarray:
    full, _ = run(inputs)
    return full
